# revision 36
# baseline (speedup 1.0000x reference)
"""Trainium2 Bass kernel for combined cross-entropy + batch-hard triplet loss.

Problem (N=4096, C=751, D=2048, 1024 identities x 4 instances):
  loss = mean(-log_softmax(logits)[i, t_i]) +
         mean(relu(max_same(dist) - min_diff(dist) + 0.5))
  with dist = pairwise Euclidean distances of feat rows.

Strategy (measured ~68us on 8 NeuronCores, ~2.8x over a straightforward
row-parallel fp32 version):

* Symmetric tiling: the distance matrix is symmetric, so only upper-triangle
  tiles are computed -- at [128-row x 512-col] granularity 144 of 256 tiles,
  18 per core, balanced by a uniform slot pattern (4 diag + 4 + 2 + 8 tiles
  over at most two distinct 512-col blocks per core; SPMD addressing is
  identical on every core, per-core differences live in the input data).
* fp8 e4m3 everywhere on the PE with DoubleRow perf mode (~1.44x over
  bf16-rate at free-dim 512).  Features are quantized to fp8 on the host;
  the exact distance identity d2 = |f8_i - f8_j|^2 then holds for the
  quantized vectors, so the only error is a tiny input perturbation
  (~2e-4 relative on the final loss, gate is 2e-2).
* Per tile the PSUM accumulates
    psum = f8_i . f8_j - sq_i/2 - sq_j/2 - 8192*same(i,j)
  via 8 K=256 DoubleRow matmuls plus one fold matmul whose 40 contraction
  rows carry the same-pair mask (64 x -128) and a 4-level fp8 decomposition
  of -sq/32 against 16.0 (exact to ~0.03 in d2).
* Mining: row-side DVE max (hardest negative) on every tile, DVE min
  (hardest positive) on the 4 diagonal tiles; column-side coverage of the
  transposed region via ACT psum->SBUF f16 copy, PE transpose, DVE max.
* Warmup: the 4 diagonal tiles' lhsT is a slice of the already-resident
  slot0 rhs, so the PE starts after ~0.5 MB of DMA and runs k-major while
  the remaining slots stream in on three queues (SP=rhs+lhs, ACT=copies,
  gpsimd=fold/logits).  DMA issues are batched (~0.6us of engine time
  each).
* Cross entropy: device computes log-sum-exp per row (ACT Exp with fused
  accumulation, then Ln) over bf16 logits; host subtracts the gathered
  target logit.
* Host combines the small partial outputs (max over row/col-side partials
  -> dist_an, diagonal min -> dist_ap, sqrt/relu/mean in f64) and adds the
  xent term.  Rows are pre-sorted by target (the loss is permutation
  invariant) so same-identity groups are 4 consecutive rows, which makes
  the mask a fixed block pattern.
"""
import sys

if "/opt/trn_rl_repo" not in sys.path:
    sys.path.insert(0, "/opt/trn_rl_repo")

import numpy as np
import ml_dtypes

BF16 = ml_dtypes.bfloat16
FP8 = ml_dtypes.float8_e4m3

N = 4096
D = 2048
C = 751
NCORES = 8
RPC = N // NCORES          # xent rows per core = 512
NT = 18                    # distance tiles per core
KT = D // 128              # K chunks = 16
KF = 36                    # fold matmul contraction size
BIG = 16384.0              # same-pair offset in q = -2*psum
MASK_L = 64.0              # lhs mask scale; 64 * 128 = BIG/2 = 8192
MASK_R = 128.0             # rhs mask scale (max finite e4m3 is 240)
ALPHA = 1.0
BETA = 1.0
MARGIN = 0.5

# --- static tile assignment -------------------------------------------------
# Kept tiles: (row_tile r in 0..31, col_block c in 0..7) with r <= 4c+3.
# Slot sizes per core: [4, 4, 2, 8]; slot0 always holds the 4 diagonal-band
# tiles (r in 4c..4c+3). Each core touches at most 2 distinct col blocks.
SLOT_SIZES = [4, 4, 2, 8]
# tile positions: t0-3 slot0 (diag, warmup), t4-7 slot1, t8-15 slot3,
# t16-17 slot2 (the 2-tile slot last: its block can arrive when the bus
# is otherwise drained)
SLOT_OF_T = [0] * 4 + [1] * 4 + [2] * 2 + [3] * 8
ASSIGN = [
    [(0, [0, 1, 2, 3]), (7, [0, 1, 2, 3]), (7, [4, 5]), (7, list(range(6, 14)))],
    [(7, [28, 29, 30, 31]), (7, [14, 15, 16, 17]), (7, [18, 19]), (7, list(range(20, 28)))],
    [(1, [4, 5, 6, 7]), (1, [0, 1, 2, 3]), (6, [0, 1]), (6, list(range(2, 10)))],
    [(6, [24, 25, 26, 27]), (6, [10, 11, 12, 13]), (6, [14, 15]), (6, list(range(16, 24)))],
    [(2, [8, 9, 10, 11]), (5, [0, 1, 2, 3]), (5, [4, 5]), (2, list(range(0, 8)))],
    [(5, [20, 21, 22, 23]), (5, [6, 7, 8, 9]), (5, [10, 11]), (5, list(range(12, 20)))],
    [(3, [12, 13, 14, 15]), (3, [0, 1, 2, 3]), (4, [0, 1]), (3, list(range(4, 12)))],
    [(4, [16, 17, 18, 19]), (4, [2, 3, 4, 5]), (4, [6, 7]), (4, list(range(8, 16)))],
]

# TILES[c] = [(row_tile, col_block)] * 18, diag tiles at t=0..3
TILES = []
for _c in range(NCORES):
    _tl = []
    for _sl in (0, 1, 2, 3):
        _cb, _rows = ASSIGN[_c][_sl]
        _tl.extend((_r, _cb) for _r in _rows)
    TILES.append(_tl)

# SLOT_BLOCK[c][s] = col block resident in slot s for core c
SLOT_BLOCK = [[cb for cb, _ in ASSIGN[c]] for c in range(NCORES)]

# TILE_AT[(r, cb)] = (core, t)
TILE_AT = {}
for _c in range(NCORES):
    for _t, (_r, _cb) in enumerate(TILES[_c]):
        assert (_r, _cb) not in TILE_AT
        TILE_AT[(_r, _cb)] = (_c, _t)

# sanity: full upper-triangle coverage, diag placement
assert len(TILE_AT) == 144
for _r in range(32):
    for _cb in range(_r // 4, 8):
        assert (_r, _cb) in TILE_AT
DIAG_T = (0, 1, 2, 3)
for _c in range(NCORES):
    for _t, (_r, _cb) in enumerate(TILES[_c]):
        assert (_t in DIAG_T) == (_r // 4 == _cb), (_c, _t, _r, _cb)
        if _t in DIAG_T:
            _g = _t if _t < 3 else 3
            assert _r == 4 * _cb + _g  # diag lhsT = slot0 rhs slice

# out1 column layout
O_RMAX = 0                 # cols 0..17  : row-side max per tile
O_RMIN = NT                # cols 18..21 : row-side min, diag tiles t=0..3
O_LSE = NT + 4             # cols 22..25 : xent log-sum-exp per 128-row tile
O_W = NT + 8               # 26 cols

_compiled = {}


def _build_nc():
    import concourse.bass as bass  # noqa: F401
    import concourse.tile as tile
    from concourse import mybir, bacc
    from contextlib import ExitStack

    f32 = mybir.dt.float32
    f16 = mybir.dt.float16
    bf16 = mybir.dt.bfloat16
    fp8 = mybir.dt.float8e4
    PM = mybir.MatmulPerfMode.DoubleRow
    Alu = mybir.AluOpType
    Act = mybir.ActivationFunctionType
    X = mybir.AxisListType.X

    nc = bacc.Bacc("TRN2", target_bir_lowering=False, debug=False)

    # rhs_pack[s][p, j*1024 + i*512 + n]: slot s, DoubleRow pair j, subtile i
    rhs_in = nc.dram_tensor("rhs_pack", [4, 128, 8192], fp8, kind="ExternalInput").ap()
    # lhs_pack[u][p, tt*2048 + j*256 + i*128 + m]: tile pair u = tiles (4+2u, 5+2u)
    lhs_in = nc.dram_tensor("lhs_pack", [(NT - 4) // 2, 128, 4096], fp8, kind="ExternalInput").ap()
    flh_in = nc.dram_tensor("fold_lhs", [20, 2, NT * 128], fp8, kind="ExternalInput").ap()
    frh_in = nc.dram_tensor("fold_rhs", [NT, 20, 2, 512], fp8, kind="ExternalInput").ap()
    logits_in = nc.dram_tensor("logits", [128, 4 * C], bf16, kind="ExternalInput").ap()
    ident_in = nc.dram_tensor("ident", [128, 128], f16, kind="ExternalInput").ap()
    out1_dram = nc.dram_tensor("out1", [128, O_W], f32, kind="ExternalOutput").ap()
    out2_dram = nc.dram_tensor("out2", [128, (NT - 4) * 4], f32, kind="ExternalOutput").ap()

    with tile.TileContext(nc) as tc, ExitStack() as ctx:
        resident = ctx.enter_context(tc.tile_pool(name="resident", bufs=1))
        lhs_pool = ctx.enter_context(tc.tile_pool(name="lhs", bufs=(NT - 4) // 2))
        fr_pool = ctx.enter_context(tc.tile_pool(name="fr", bufs=NT))
        sb_pool = ctx.enter_context(tc.tile_pool(name="sbt", bufs=4))
        ps_pool = ctx.enter_context(tc.tile_pool(name="ps", bufs=6, space="PSUM"))
        pt_pool = ctx.enter_context(tc.tile_pool(name="pt", bufs=2, space="PSUM"))
        xent_pool = ctx.enter_context(tc.tile_pool(name="xent", bufs=2))
        small_pool = ctx.enter_context(tc.tile_pool(name="small", bufs=4))

        # --- resident loads.  SP queue carries rhs + lhs interleaved in
        # consumption order (warmup diag tiles read ONLY slot0 -- their lhsT
        # is a slice of the same resident data); gpsimd (SWDGE) carries
        # fold/logits/identity.  Few large DMAs: each dma_start costs the
        # issuing engine ~0.6us of descriptor generation.
        # slot0 in 4 groups of 2 chunk-pairs (fine warmup deps)
        R0_SIZES = (2, 2, 2, 2)
        r0g = []
        for g, npair in enumerate(R0_SIZES):
            rg = resident.tile([128, npair, 2, 512], fp8, tag=f"r0g{g}",
                               name=f"r0g{g}")
            r0g.append(rg)
        rs = [None] * 4   # slots 1..3 (slot1 in two halves)
        for ss in range(1, 4):
            rt = resident.tile([128, KT // 2, 2, 512], fp8, tag=f"rs{ss}",
                               name=f"rs{ss}")
            rs[ss] = rt
        rs1b = resident.tile([128, KT // 4, 2, 512], fp8, name="rs1b")
        lhs_pairs = []
        for u in range((NT - 4) // 2):
            lt = lhs_pool.tile([128, 2, KT // 2, 2, 128], fp8, tag="lhs")
            lhs_pairs.append(lt)

        # SP-queue emission order = arrival order: interleave the next slot
        # between slot0 warmup groups -- the (DMA-paced) warmup absorbs the
        # latency of rs1 loading behind it
        nc.sync.dma_start(r0g[0][:], rhs_in[0][:, 0:2048])
        nc.sync.dma_start(r0g[1][:], rhs_in[0][:, 2048:4096])
        nc.sync.dma_start(rs[1][:, 0:4], rhs_in[1][:, 0:4096])
        nc.sync.dma_start(r0g[2][:], rhs_in[0][:, 4096:6144])
        nc.sync.dma_start(rs1b[:], rhs_in[1][:, 4096:8192])
        nc.sync.dma_start(r0g[3][:], rhs_in[0][:, 6144:8192])
        nc.sync.dma_start(lhs_pairs[0][:], lhs_in[0])
        nc.sync.dma_start(lhs_pairs[1][:], lhs_in[1])
        nc.sync.dma_start(rs[2][:], rhs_in[2])
        nc.sync.dma_start(lhs_pairs[2][:], lhs_in[2])
        # rs3 pulls in parallel on the otherwise-idle ACT HWDGE queue
        nc.scalar.dma_start(rs[3][:], rhs_in[3])
        for u in range(3, (NT - 4) // 2):
            nc.sync.dma_start(lhs_pairs[u][:], lhs_in[u])

        ident = resident.tile([128, 128], f16)
        nc.gpsimd.dma_start(ident[:], ident_in[:])
        flh = resident.tile([20, 2, NT * 128], fp8)
        nc.gpsimd.dma_start(flh[:], flh_in[:])
        fr_tiles = []
        for t in range(NT):
            fr = fr_pool.tile([20, 2, 512], fp8, tag="fr")
            nc.gpsimd.dma_start(fr[:], frh_in[t])
            fr_tiles.append(fr)
        lg_all = resident.tile([128, 4 * C], bf16)
        nc.gpsimd.dma_start(lg_all[:], logits_in[:])

        out_tile = resident.tile([128, O_W], f32)
        out2_tile = resident.tile([128, (NT - 4) * 4], f32)

        R0_OF_J = []   # j -> (group, index within group)
        for g, npair in enumerate(R0_SIZES):
            R0_OF_J.extend((g, i) for i in range(npair))

        def rhs_op(s, j):
            if s == 0:
                g, i = R0_OF_J[j]
                return r0g[g][:, i]
            if s == 1 and j >= 4:
                return rs1b[:, j - 4]
            return rs[s][:, j]

        def diag_lhsT(j, g):
            gg, i = R0_OF_J[j]
            return r0g[gg][:, i, :, bass.ts(g, 128)]

        # --- warmup: k-major over the 4 diagonal tiles (slot0 only) ---
        ps_list = [ps_pool.tile([128, 512], f32, tag="ps", name=f"psw{_t}")
                   for _t in range(4)]
        for j in range(KT // 2):
            for t in range(4):
                nc.tensor.matmul(
                    ps_list[t][:],
                    diag_lhsT(j, t),
                    rhs_op(0, j),
                    start=(j == 0),
                    stop=False,
                    perf_mode=PM,
                )
        for t in range(4):
            nc.tensor.matmul(ps_list[t][:], flh[:, :, bass.ts(t, 128)],
                             fr_tiles[t][:], start=False, stop=True, perf_mode=PM)
            nc.vector.tensor_reduce(out_tile[:, O_RMAX + t:O_RMAX + t + 1],
                                    ps_list[t][:], axis=X, op=Alu.max)
            nc.vector.tensor_reduce(out_tile[:, O_RMIN + t:O_RMIN + t + 1],
                                    ps_list[t][:], axis=X, op=Alu.min)

        # --- cross entropy: lse per row (grouped per activation function
        # to avoid ACT table reloads) ---
        saccs = []
        for x in range(4):
            escr = xent_pool.tile([128, C], bf16, tag="escr")
            sacc = small_pool.tile([128, 1], f32, tag="s")
            nc.scalar.activation(escr[:], lg_all[:, bass.ts(x, C)], Act.Exp,
                                 accum_out=sacc[:])
            saccs.append(sacc)
        for x in range(4):
            nc.scalar.activation(out_tile[:, O_LSE + x:O_LSE + x + 1], saccs[x][:],
                                 Act.Ln, scale=1.0)

        # --- main loop: tiles 4..17, software-pipelined transposes ---
        sb_tiles = [None] * NT

        def emit_transpose(t):
            sb = sb_tiles[t]
            pt = pt_pool.tile([128, 4, 128], f16, tag="pt")
            for i in range(4):
                nc.tensor.transpose(pt[:, i, :], sb[:, bass.ts(i, 128)], ident[:])
            nc.vector.tensor_reduce(out2_tile[:, bass.ts(t - 4, 4)], pt[:],
                                    axis=X, op=Alu.max)

        # tiles processed in pairs with interleaved matmul streams: the PE
        # reorder window can overlap the two banks' weight loads
        for ta in range(4, NT, 2):
            tb = ta + 1
            psa = ps_pool.tile([128, 512], f32, tag="ps", name=f"psa{ta}")
            psb = ps_pool.tile([128, 512], f32, tag="ps", name=f"psb{ta}")
            for j in range(KT // 2):
                for t, ps in ((ta, psa), (tb, psb)):
                    lhsT = lhs_pairs[(t - 4) // 2][:, (t - 4) % 2, j]
                    nc.tensor.matmul(ps[:], lhsT, rhs_op(SLOT_OF_T[t], j),
                                     start=(j == 0), stop=False, perf_mode=PM)
            for t, ps in ((ta, psa), (tb, psb)):
                nc.tensor.matmul(ps[:], flh[:, :, bass.ts(t, 128)],
                                 fr_tiles[t][:], start=False, stop=True,
                                 perf_mode=PM)
            for t, ps in ((ta, psa), (tb, psb)):
                nc.vector.tensor_reduce(out_tile[:, O_RMAX + t:O_RMAX + t + 1],
                                        ps[:], axis=X, op=Alu.max)
                sb = sb_pool.tile([128, 512], f16, tag="sb")
                nc.scalar.activation(sb[:], ps[:], Act.Copy, scale=1.0)
                sb_tiles[t] = sb
            # transposes of the previous pair land behind this pair's matmuls
            if ta >= 6:
                emit_transpose(ta - 2)
                emit_transpose(ta - 1)
        emit_transpose(NT - 2)
        emit_transpose(NT - 1)

        nc.sync.dma_start(out2_dram[:], out2_tile[:])
        nc.sync.dma_start(out1_dram[:], out_tile[:])

    nc.compile()
    return nc


def _prepare(logits, feat, targets):
    logits = np.asarray(logits, dtype=np.float32)
    feat = np.asarray(feat, dtype=np.float32)
    targets = np.asarray(targets)

    perm = np.argsort(targets, kind="stable")
    t_sorted = np.asarray(targets)[perm]
    tg = t_sorted.reshape(-1, 4)
    assert (tg == tg[:, :1]).all(), "expected PK sampling with 4 instances/identity"

    F = feat[perm].astype(FP8)                   # [N, D] fp8 e4m3
    FT = np.ascontiguousarray(F.T)               # [D, N] fp8
    F64 = F.astype(np.float64)
    sq = np.einsum("ij,ij->i", F64, F64).astype(np.float32)
    # 4-level fp8 decomposition of -sq/32 (e4m3 max is 448; the factor 16
    # sits on the "ones" side so each product contributes -sq/2 overall)
    sq_lv = []
    res = (sq / -32.0).astype(np.float32)
    for _ in range(4):
        lv = res.astype(FP8).astype(np.float32)
        sq_lv.append(lv)
        res = res - lv

    logits_p = logits[perm].astype(BF16)

    # mask patterns (bf16-exact values, stored f32 then cast)
    mask_lhs = np.zeros((32, 128), dtype=np.float32)
    m_idx = np.arange(128)
    mask_lhs[m_idx // 4, m_idx] = MASK_L

    FT3 = FT.reshape(KT, 128, N)
    FT4 = FT.reshape(KT // 2, 2, 128, N)

    in_maps = []
    for c in range(NCORES):
        tiles = TILES[c]

        # rhs_pack [4, 128, 8192]: [s][p, j*1024 + i*512 + n],
        # slot s holds block SLOT_BLOCK[c][s]
        rhs_pack = np.empty((4, 128, 8192), dtype=FP8)
        for sl in range(4):
            cb0 = 512 * SLOT_BLOCK[c][sl]
            blk = FT4[:, :, :, cb0:cb0 + 512]           # [KT//2 j, 2 i, 128p, 512]
            rhs_pack[sl] = blk.transpose(2, 0, 1, 3).reshape(128, 8192)

        # lhs_pack [(NT-4)//2, 128, 4096]: tile pair u = (4+2u, 5+2u),
        # [u][p, tt*2048 + j*256 + i*128 + m] = FT[128*(2j+i)+p, rows_t[m]]
        # (diag tiles t<4 read their lhsT out of the resident slot0 rhs)
        lhs_pack = np.empty(((NT - 4) // 2, 128, 4096), dtype=FP8)
        for t, (r, _cb) in enumerate(tiles):
            if t < 4:
                continue
            blk = FT3[:, :, 128 * r:128 * r + 128]      # [KT, 128p, 128m]
            u, tt = (t - 4) // 2, (t - 4) % 2
            lhs_pack[u][:, 2048 * tt:2048 * (tt + 1)] = \
                blk.transpose(1, 0, 2).reshape(128, 2048)

        # fp8 fold: 40 logical contraction rows = 32 mask + 4 row-sq levels
        # + 4 col-sq levels, packed as DoubleRow [20, 2, .] (row k -> (k%20,
        # k//20)).  sq levels are a 4-term fp8 decomposition of -sq/2 (exact
        # to ~0.016).
        flh40 = np.zeros((40, NT * 128), dtype=np.float32)
        frh40 = np.zeros((NT, 40, 512), dtype=np.float32)
        for t, (r, cb) in enumerate(tiles):
            cs = slice(128 * t, 128 * t + 128)
            flh40[:32, cs] = mask_lhs
            rows = slice(128 * r, 128 * r + 128)
            for lv in range(4):
                flh40[32 + lv, cs] = sq_lv[lv][rows]     # row-sq levels
                flh40[36 + lv, cs] = 16.0                # x16 for col-sq
            cols_blk = slice(512 * cb, 512 * cb + 512)
            for lv in range(4):
                frh40[t, 32 + lv] = 16.0                 # x16 for row-sq
                frh40[t, 36 + lv] = sq_lv[lv][cols_blk]  # col-sq levels
            if r // 4 == cb:  # diagonal-band tile: same-pair mask
                base = 128 * (r % 4)
                for g in range(32):
                    frh40[t, g, base + 4 * g: base + 4 * g + 4] = -MASK_R
        flh = flh40.reshape(2, 20, NT * 128).transpose(1, 0, 2)
        frh = frh40.reshape(NT, 2, 20, 512).transpose(0, 2, 1, 3)

        lgp = logits_p[c * RPC:(c + 1) * RPC]       # [512, C] bf16
        lg_pack = np.ascontiguousarray(
            lgp.reshape(4, 128, C).transpose(1, 0, 2).reshape(128, 4 * C))

        in_maps.append({
            "rhs_pack": rhs_pack,
            "lhs_pack": lhs_pack,
            "fold_lhs": np.ascontiguousarray(flh).astype(FP8),
            "fold_rhs": np.ascontiguousarray(frh).astype(FP8),
            "logits": lg_pack,
            "ident": np.eye(128, dtype=np.float16),
        })

    # stash for _combine
    _prepare.cache = {
        "logits_p_bf": logits_p.astype(np.float64),
        "t_sorted": t_sorted,
    }
    return in_maps


def _combine(results):
    cache = _prepare.cache
    out1 = [np.asarray(r["out1"], dtype=np.float64) for r in results]
    out2 = [np.asarray(r["out2"], dtype=np.float64) for r in results]

    # --- triplet ---
    qmax = np.empty(N)
    qmin = np.empty(N)
    for rt in range(32):
        R = rt // 4
        rows = slice(128 * rt, 128 * rt + 128)
        parts = []
        for cb in range(R, 8):                       # row-side partials
            c, t = TILE_AT[(rt, cb)]
            parts.append(out1[c][:, O_RMAX + t])
        for rp in range(0, 4 * R):                   # col-side partials
            c, t = TILE_AT[(rp, R)]
            parts.append(out2[c][:, 4 * (t - 4) + rt - 4 * R])
        qmax[rows] = np.max(np.stack(parts), axis=0)
        c, t = TILE_AT[(rt, R)]
        qmin[rows] = out1[c][:, O_RMIN + rt - 4 * R]

    d2_an = np.maximum(-2.0 * qmax, 1e-12)
    d2_ap = np.maximum(-2.0 * qmin - BIG, 1e-12)
    dist_an = np.sqrt(d2_an)
    dist_ap = np.sqrt(d2_ap)
    trip = np.mean(np.maximum(dist_ap - dist_an + MARGIN, 0.0))

    # --- cross entropy ---
    lse = np.empty(N)
    for c in range(NCORES):
        for x in range(4):
            lse[c * RPC + 128 * x: c * RPC + 128 * (x + 1)] = \
                out1[c][:, O_LSE + x]
    ti = cache["t_sorted"].astype(np.int64)
    ti = np.where(ti < 0, ti + C, ti)
    ti = np.clip(ti, 0, C - 1)
    gathered = cache["logits_p_bf"][np.arange(N), ti]
    xent = np.mean(lse - gathered)

    return np.float32(ALPHA * xent + BETA * trip)


def kernel(logits, feat, targets):
    from concourse.bass_utils import run_bass_kernel_spmd

    if "nc" not in _compiled:
        _compiled["nc"] = _build_nc()
    nc = _compiled["nc"]

    in_maps = _prepare(logits, feat, targets)
    res = run_bass_kernel_spmd(nc, in_maps, core_ids=list(range(NCORES)))
    return _combine(res.results)


# revision 37
# speedup vs baseline: 1.0409x; 1.0409x over previous
"""Trainium2 Bass kernel for combined cross-entropy + batch-hard triplet loss.

Problem (N=4096, C=751, D=2048, 1024 identities x 4 instances):
  loss = mean(-log_softmax(logits)[i, t_i]) +
         mean(relu(max_same(dist) - min_diff(dist) + 0.5))
  with dist = pairwise Euclidean distances of feat rows.

Strategy (measured ~68us on 8 NeuronCores, ~2.8x over a straightforward
row-parallel fp32 version):

* Symmetric tiling: the distance matrix is symmetric, so only upper-triangle
  tiles are computed -- at [128-row x 512-col] granularity 144 of 256 tiles,
  18 per core, balanced by a uniform slot pattern (4 diag + 4 + 2 + 8 tiles
  over at most two distinct 512-col blocks per core; SPMD addressing is
  identical on every core, per-core differences live in the input data).
* fp8 e4m3 everywhere on the PE with DoubleRow perf mode (~1.44x over
  bf16-rate at free-dim 512).  Features are quantized to fp8 on the host;
  the exact distance identity d2 = |f8_i - f8_j|^2 then holds for the
  quantized vectors, so the only error is a tiny input perturbation
  (~2e-4 relative on the final loss, gate is 2e-2).
* Per tile the PSUM accumulates
    psum = f8_i . f8_j - sq_i/2 - sq_j/2 - 8192*same(i,j)
  via 8 K=256 DoubleRow matmuls plus one fold matmul whose 40 contraction
  rows carry the same-pair mask (64 x -128) and a 4-level fp8 decomposition
  of -sq/32 against 16.0 (exact to ~0.03 in d2).
* Mining: row-side DVE max (hardest negative) on every tile, DVE min
  (hardest positive) on the 4 diagonal tiles; column-side coverage of the
  transposed region via ACT psum->SBUF f16 copy, PE transpose, DVE max.
* Warmup: the 4 diagonal tiles' lhsT is a slice of the already-resident
  slot0 rhs, so the PE starts after ~0.5 MB of DMA and runs k-major while
  the remaining slots stream in on three queues (SP=rhs+lhs, ACT=copies,
  gpsimd=fold/logits).  DMA issues are batched (~0.6us of engine time
  each).
* Cross entropy: device computes log-sum-exp per row (ACT Exp with fused
  accumulation, then Ln) over bf16 logits; host subtracts the gathered
  target logit.
* Host combines the small partial outputs (max over row/col-side partials
  -> dist_an, diagonal min -> dist_ap, sqrt/relu/mean in f64) and adds the
  xent term.  Rows are pre-sorted by target (the loss is permutation
  invariant) so same-identity groups are 4 consecutive rows, which makes
  the mask a fixed block pattern.
"""
import sys

if "/opt/trn_rl_repo" not in sys.path:
    sys.path.insert(0, "/opt/trn_rl_repo")

import numpy as np
import ml_dtypes

BF16 = ml_dtypes.bfloat16
FP8 = ml_dtypes.float8_e4m3

N = 4096
D = 2048
C = 751
NCORES = 8
RPC = N // NCORES          # xent rows per core = 512
NT = 18                    # distance tiles per core
KT = D // 128              # K chunks = 16
KF = 36                    # fold matmul contraction size
BIG = 16384.0              # same-pair offset in q = -2*psum
MASK_L = 64.0              # lhs mask scale; 64 * 128 = BIG/2 = 8192
MASK_R = 128.0             # rhs mask scale (max finite e4m3 is 240)
ALPHA = 1.0
BETA = 1.0
MARGIN = 0.5

# --- static tile assignment -------------------------------------------------
# Kept tiles: (row_tile r in 0..31, col_block c in 0..7) with r <= 4c+3.
# Slot sizes per core: [4, 4, 2, 8]; slot0 always holds the 4 diagonal-band
# tiles (r in 4c..4c+3). Each core touches at most 2 distinct col blocks.
SLOT_SIZES = [4, 4, 2, 8]
# tile positions: t0-3 slot0 (diag, warmup), t4-7 slot1, t8-15 slot3,
# t16-17 slot2 (the 2-tile slot last: its block can arrive when the bus
# is otherwise drained)
SLOT_OF_T = [0] * 4 + [1] * 4 + [2] * 2 + [3] * 8
ASSIGN = [
    [(0, [0, 1, 2, 3]), (7, [0, 1, 2, 3]), (7, [4, 5]), (7, list(range(6, 14)))],
    [(7, [28, 29, 30, 31]), (7, [14, 15, 16, 17]), (7, [18, 19]), (7, list(range(20, 28)))],
    [(1, [4, 5, 6, 7]), (1, [0, 1, 2, 3]), (6, [0, 1]), (6, list(range(2, 10)))],
    [(6, [24, 25, 26, 27]), (6, [10, 11, 12, 13]), (6, [14, 15]), (6, list(range(16, 24)))],
    [(2, [8, 9, 10, 11]), (5, [0, 1, 2, 3]), (5, [4, 5]), (2, list(range(0, 8)))],
    [(5, [20, 21, 22, 23]), (5, [6, 7, 8, 9]), (5, [10, 11]), (5, list(range(12, 20)))],
    [(3, [12, 13, 14, 15]), (3, [0, 1, 2, 3]), (4, [0, 1]), (3, list(range(4, 12)))],
    [(4, [16, 17, 18, 19]), (4, [2, 3, 4, 5]), (4, [6, 7]), (4, list(range(8, 16)))],
]

# TILES[c] = [(row_tile, col_block)] * 18, diag tiles at t=0..3
TILES = []
for _c in range(NCORES):
    _tl = []
    for _sl in (0, 1, 2, 3):
        _cb, _rows = ASSIGN[_c][_sl]
        _tl.extend((_r, _cb) for _r in _rows)
    TILES.append(_tl)

# SLOT_BLOCK[c][s] = col block resident in slot s for core c
SLOT_BLOCK = [[cb for cb, _ in ASSIGN[c]] for c in range(NCORES)]

# TILE_AT[(r, cb)] = (core, t)
TILE_AT = {}
for _c in range(NCORES):
    for _t, (_r, _cb) in enumerate(TILES[_c]):
        assert (_r, _cb) not in TILE_AT
        TILE_AT[(_r, _cb)] = (_c, _t)

# sanity: full upper-triangle coverage, diag placement
assert len(TILE_AT) == 144
for _r in range(32):
    for _cb in range(_r // 4, 8):
        assert (_r, _cb) in TILE_AT
DIAG_T = (0, 1, 2, 3)
for _c in range(NCORES):
    for _t, (_r, _cb) in enumerate(TILES[_c]):
        assert (_t in DIAG_T) == (_r // 4 == _cb), (_c, _t, _r, _cb)
        if _t in DIAG_T:
            _g = _t if _t < 3 else 3
            assert _r == 4 * _cb + _g  # diag lhsT = slot0 rhs slice

# out1 column layout
O_RMAX = 0                 # cols 0..17  : row-side max per tile
O_RMIN = NT                # cols 18..21 : row-side min, diag tiles t=0..3
O_LSE = NT + 4             # cols 22..25 : xent log-sum-exp per 128-row tile
O_W = NT + 8               # 26 cols

_compiled = {}


def _build_nc():
    import concourse.bass as bass  # noqa: F401
    import concourse.tile as tile
    from concourse import mybir, bacc
    from contextlib import ExitStack

    f32 = mybir.dt.float32
    f16 = mybir.dt.float16
    bf16 = mybir.dt.bfloat16
    fp8 = mybir.dt.float8e4
    PM = mybir.MatmulPerfMode.DoubleRow
    Alu = mybir.AluOpType
    Act = mybir.ActivationFunctionType
    X = mybir.AxisListType.X

    nc = bacc.Bacc("TRN2", target_bir_lowering=False, debug=False)

    # rhs_pack[s][p, j*1024 + i*512 + n]: slot s, DoubleRow pair j, subtile i
    rhs_in = nc.dram_tensor("rhs_pack", [4, 128, 8192], fp8, kind="ExternalInput").ap()
    # lhs_pack[u][p, tt*2048 + j*256 + i*128 + m]: tile pair u = tiles (4+2u, 5+2u)
    lhs_in = nc.dram_tensor("lhs_pack", [(NT - 4) // 2, 128, 4096], fp8, kind="ExternalInput").ap()
    flh_in = nc.dram_tensor("fold_lhs", [20, 2, NT * 128], fp8, kind="ExternalInput").ap()
    frh_in = nc.dram_tensor("fold_rhs", [NT, 20, 2, 512], fp8, kind="ExternalInput").ap()
    logits_in = nc.dram_tensor("logits", [128, 4 * C], bf16, kind="ExternalInput").ap()
    ident_in = nc.dram_tensor("ident", [128, 128], f16, kind="ExternalInput").ap()
    out1_dram = nc.dram_tensor("out1", [128, O_W], f32, kind="ExternalOutput").ap()
    out2_dram = nc.dram_tensor("out2", [128, (NT - 4) * 4], f32, kind="ExternalOutput").ap()

    with tile.TileContext(nc) as tc, ExitStack() as ctx:
        resident = ctx.enter_context(tc.tile_pool(name="resident", bufs=1))
        lhs_pool = ctx.enter_context(tc.tile_pool(name="lhs", bufs=(NT - 4) // 2))
        fr_pool = ctx.enter_context(tc.tile_pool(name="fr", bufs=NT))
        sb_pool = ctx.enter_context(tc.tile_pool(name="sbt", bufs=4))
        ps_pool = ctx.enter_context(tc.tile_pool(name="ps", bufs=6, space="PSUM"))
        pt_pool = ctx.enter_context(tc.tile_pool(name="pt", bufs=2, space="PSUM"))
        xent_pool = ctx.enter_context(tc.tile_pool(name="xent", bufs=2))
        small_pool = ctx.enter_context(tc.tile_pool(name="small", bufs=4))

        # --- resident loads.  SP queue carries rhs + lhs interleaved in
        # consumption order (warmup diag tiles read ONLY slot0 -- their lhsT
        # is a slice of the same resident data); gpsimd (SWDGE) carries
        # fold/logits/identity.  Few large DMAs: each dma_start costs the
        # issuing engine ~0.6us of descriptor generation.
        # slot0 in 4 groups of 2 chunk-pairs (fine warmup deps)
        R0_SIZES = (2, 2, 2, 2)
        r0g = []
        for g, npair in enumerate(R0_SIZES):
            rg = resident.tile([128, npair, 2, 512], fp8, tag=f"r0g{g}",
                               name=f"r0g{g}")
            r0g.append(rg)
        rs = [None] * 4   # slots 1..3 (slot1 in two halves)
        for ss in range(1, 4):
            rt = resident.tile([128, KT // 2, 2, 512], fp8, tag=f"rs{ss}",
                               name=f"rs{ss}")
            rs[ss] = rt
        rs1b = resident.tile([128, KT // 4, 2, 512], fp8, name="rs1b")
        rs3b = resident.tile([128, KT // 4, 2, 512], fp8, name="rs3b")
        lhs_pairs = []
        for u in range((NT - 4) // 2):
            lt = lhs_pool.tile([128, 2, KT // 2, 2, 128], fp8, tag="lhs")
            lhs_pairs.append(lt)

        # SP-queue emission order = arrival order: interleave the next slot
        # between slot0 warmup groups -- the (DMA-paced) warmup absorbs the
        # latency of rs1 loading behind it
        nc.sync.dma_start(r0g[0][:], rhs_in[0][:, 0:2048])
        nc.sync.dma_start(r0g[1][:], rhs_in[0][:, 2048:4096])
        nc.sync.dma_start(rs[1][:, 0:4], rhs_in[1][:, 0:4096])
        nc.sync.dma_start(r0g[2][:], rhs_in[0][:, 4096:6144])
        nc.sync.dma_start(rs1b[:], rhs_in[1][:, 4096:8192])
        nc.sync.dma_start(r0g[3][:], rhs_in[0][:, 6144:8192])
        nc.sync.dma_start(lhs_pairs[0][:], lhs_in[0])
        nc.sync.dma_start(lhs_pairs[1][:], lhs_in[1])
        nc.sync.dma_start(rs[2][:], rhs_in[2])
        nc.sync.dma_start(lhs_pairs[2][:], lhs_in[2])
        nc.sync.dma_start(rs[3][:, 0:4], rhs_in[3][:, 0:4096])
        nc.sync.dma_start(lhs_pairs[3][:], lhs_in[3])
        nc.sync.dma_start(rs3b[:], rhs_in[3][:, 4096:8192])
        for u in range(4, (NT - 4) // 2):
            nc.sync.dma_start(lhs_pairs[u][:], lhs_in[u])

        ident = resident.tile([128, 128], f16)
        nc.gpsimd.dma_start(ident[:], ident_in[:])
        flh = resident.tile([20, 2, NT * 128], fp8)
        nc.gpsimd.dma_start(flh[:], flh_in[:])
        fr_tiles = []
        for t in range(NT):
            fr = fr_pool.tile([20, 2, 512], fp8, tag="fr")
            nc.gpsimd.dma_start(fr[:], frh_in[t])
            fr_tiles.append(fr)
        lg_all = resident.tile([128, 4 * C], bf16)
        nc.gpsimd.dma_start(lg_all[:], logits_in[:])

        out_tile = resident.tile([128, O_W], f32)
        out2_tile = resident.tile([128, (NT - 4) * 4], f32)

        R0_OF_J = []   # j -> (group, index within group)
        for g, npair in enumerate(R0_SIZES):
            R0_OF_J.extend((g, i) for i in range(npair))

        def rhs_op(s, j):
            if s == 0:
                g, i = R0_OF_J[j]
                return r0g[g][:, i]
            if s == 1 and j >= 4:
                return rs1b[:, j - 4]
            if s == 3 and j >= 4:
                return rs3b[:, j - 4]
            return rs[s][:, j]

        def diag_lhsT(j, g):
            gg, i = R0_OF_J[j]
            return r0g[gg][:, i, :, bass.ts(g, 128)]

        # --- warmup: k-major over the 4 diagonal tiles (slot0 only) ---
        ps_list = [ps_pool.tile([128, 512], f32, tag="ps", name=f"psw{_t}")
                   for _t in range(4)]
        for j in range(KT // 2):
            for t in range(4):
                nc.tensor.matmul(
                    ps_list[t][:],
                    diag_lhsT(j, t),
                    rhs_op(0, j),
                    start=(j == 0),
                    stop=False,
                    perf_mode=PM,
                )
        for t in range(4):
            nc.tensor.matmul(ps_list[t][:], flh[:, :, bass.ts(t, 128)],
                             fr_tiles[t][:], start=False, stop=True, perf_mode=PM)
            nc.vector.tensor_reduce(out_tile[:, O_RMAX + t:O_RMAX + t + 1],
                                    ps_list[t][:], axis=X, op=Alu.max)
            nc.vector.tensor_reduce(out_tile[:, O_RMIN + t:O_RMIN + t + 1],
                                    ps_list[t][:], axis=X, op=Alu.min)

        # --- cross entropy: lse per row (grouped per activation function
        # to avoid ACT table reloads) ---
        saccs = []
        for x in range(4):
            escr = xent_pool.tile([128, C], bf16, tag="escr")
            sacc = small_pool.tile([128, 1], f32, tag="s")
            nc.scalar.activation(escr[:], lg_all[:, bass.ts(x, C)], Act.Exp,
                                 accum_out=sacc[:])
            saccs.append(sacc)
        for x in range(4):
            nc.scalar.activation(out_tile[:, O_LSE + x:O_LSE + x + 1], saccs[x][:],
                                 Act.Ln, scale=1.0)

        # --- main loop: tiles 4..17, software-pipelined transposes ---
        sb_tiles = [None] * NT

        def emit_transpose(t):
            sb = sb_tiles[t]
            pt = pt_pool.tile([128, 4, 128], f16, tag="pt")
            for i in range(4):
                nc.tensor.transpose(pt[:, i, :], sb[:, bass.ts(i, 128)], ident[:])
            nc.vector.tensor_reduce(out2_tile[:, bass.ts(t - 4, 4)], pt[:],
                                    axis=X, op=Alu.max)

        # tiles processed in pairs with interleaved matmul streams: the PE
        # reorder window can overlap the two banks' weight loads
        for ta in range(4, NT, 2):
            tb = ta + 1
            psa = ps_pool.tile([128, 512], f32, tag="ps", name=f"psa{ta}")
            psb = ps_pool.tile([128, 512], f32, tag="ps", name=f"psb{ta}")
            for j in range(KT // 2):
                for t, ps in ((ta, psa), (tb, psb)):
                    lhsT = lhs_pairs[(t - 4) // 2][:, (t - 4) % 2, j]
                    nc.tensor.matmul(ps[:], lhsT, rhs_op(SLOT_OF_T[t], j),
                                     start=(j == 0), stop=False, perf_mode=PM)
            for t, ps in ((ta, psa), (tb, psb)):
                nc.tensor.matmul(ps[:], flh[:, :, bass.ts(t, 128)],
                                 fr_tiles[t][:], start=False, stop=True,
                                 perf_mode=PM)
            for t, ps in ((ta, psa), (tb, psb)):
                nc.vector.tensor_reduce(out_tile[:, O_RMAX + t:O_RMAX + t + 1],
                                        ps[:], axis=X, op=Alu.max)
                sb = sb_pool.tile([128, 512], f16, tag="sb")
                nc.scalar.activation(sb[:], ps[:], Act.Copy, scale=1.0)
                sb_tiles[t] = sb
            # transposes of the previous pair land behind this pair's matmuls
            if ta >= 6:
                emit_transpose(ta - 2)
                emit_transpose(ta - 1)
        emit_transpose(NT - 2)
        emit_transpose(NT - 1)

        nc.sync.dma_start(out2_dram[:], out2_tile[:])
        nc.sync.dma_start(out1_dram[:], out_tile[:])

    nc.compile()
    return nc


def _prepare(logits, feat, targets):
    logits = np.asarray(logits, dtype=np.float32)
    feat = np.asarray(feat, dtype=np.float32)
    targets = np.asarray(targets)

    perm = np.argsort(targets, kind="stable")
    t_sorted = np.asarray(targets)[perm]
    tg = t_sorted.reshape(-1, 4)
    assert (tg == tg[:, :1]).all(), "expected PK sampling with 4 instances/identity"

    F = feat[perm].astype(FP8)                   # [N, D] fp8 e4m3
    FT = np.ascontiguousarray(F.T)               # [D, N] fp8
    F64 = F.astype(np.float64)
    sq = np.einsum("ij,ij->i", F64, F64).astype(np.float32)
    # 4-level fp8 decomposition of -sq/32 (e4m3 max is 448; the factor 16
    # sits on the "ones" side so each product contributes -sq/2 overall)
    sq_lv = []
    res = (sq / -32.0).astype(np.float32)
    for _ in range(4):
        lv = res.astype(FP8).astype(np.float32)
        sq_lv.append(lv)
        res = res - lv

    logits_p = logits[perm].astype(BF16)

    # mask patterns (bf16-exact values, stored f32 then cast)
    mask_lhs = np.zeros((32, 128), dtype=np.float32)
    m_idx = np.arange(128)
    mask_lhs[m_idx // 4, m_idx] = MASK_L

    FT3 = FT.reshape(KT, 128, N)
    FT4 = FT.reshape(KT // 2, 2, 128, N)

    in_maps = []
    for c in range(NCORES):
        tiles = TILES[c]

        # rhs_pack [4, 128, 8192]: [s][p, j*1024 + i*512 + n],
        # slot s holds block SLOT_BLOCK[c][s]
        rhs_pack = np.empty((4, 128, 8192), dtype=FP8)
        for sl in range(4):
            cb0 = 512 * SLOT_BLOCK[c][sl]
            blk = FT4[:, :, :, cb0:cb0 + 512]           # [KT//2 j, 2 i, 128p, 512]
            rhs_pack[sl] = blk.transpose(2, 0, 1, 3).reshape(128, 8192)

        # lhs_pack [(NT-4)//2, 128, 4096]: tile pair u = (4+2u, 5+2u),
        # [u][p, tt*2048 + j*256 + i*128 + m] = FT[128*(2j+i)+p, rows_t[m]]
        # (diag tiles t<4 read their lhsT out of the resident slot0 rhs)
        lhs_pack = np.empty(((NT - 4) // 2, 128, 4096), dtype=FP8)
        for t, (r, _cb) in enumerate(tiles):
            if t < 4:
                continue
            blk = FT3[:, :, 128 * r:128 * r + 128]      # [KT, 128p, 128m]
            u, tt = (t - 4) // 2, (t - 4) % 2
            lhs_pack[u][:, 2048 * tt:2048 * (tt + 1)] = \
                blk.transpose(1, 0, 2).reshape(128, 2048)

        # fp8 fold: 40 logical contraction rows = 32 mask + 4 row-sq levels
        # + 4 col-sq levels, packed as DoubleRow [20, 2, .] (row k -> (k%20,
        # k//20)).  sq levels are a 4-term fp8 decomposition of -sq/2 (exact
        # to ~0.016).
        flh40 = np.zeros((40, NT * 128), dtype=np.float32)
        frh40 = np.zeros((NT, 40, 512), dtype=np.float32)
        for t, (r, cb) in enumerate(tiles):
            cs = slice(128 * t, 128 * t + 128)
            flh40[:32, cs] = mask_lhs
            rows = slice(128 * r, 128 * r + 128)
            for lv in range(4):
                flh40[32 + lv, cs] = sq_lv[lv][rows]     # row-sq levels
                flh40[36 + lv, cs] = 16.0                # x16 for col-sq
            cols_blk = slice(512 * cb, 512 * cb + 512)
            for lv in range(4):
                frh40[t, 32 + lv] = 16.0                 # x16 for row-sq
                frh40[t, 36 + lv] = sq_lv[lv][cols_blk]  # col-sq levels
            if r // 4 == cb:  # diagonal-band tile: same-pair mask
                base = 128 * (r % 4)
                for g in range(32):
                    frh40[t, g, base + 4 * g: base + 4 * g + 4] = -MASK_R
        flh = flh40.reshape(2, 20, NT * 128).transpose(1, 0, 2)
        frh = frh40.reshape(NT, 2, 20, 512).transpose(0, 2, 1, 3)

        lgp = logits_p[c * RPC:(c + 1) * RPC]       # [512, C] bf16
        lg_pack = np.ascontiguousarray(
            lgp.reshape(4, 128, C).transpose(1, 0, 2).reshape(128, 4 * C))

        in_maps.append({
            "rhs_pack": rhs_pack,
            "lhs_pack": lhs_pack,
            "fold_lhs": np.ascontiguousarray(flh).astype(FP8),
            "fold_rhs": np.ascontiguousarray(frh).astype(FP8),
            "logits": lg_pack,
            "ident": np.eye(128, dtype=np.float16),
        })

    # stash for _combine
    _prepare.cache = {
        "logits_p_bf": logits_p.astype(np.float64),
        "t_sorted": t_sorted,
    }
    return in_maps


def _combine(results):
    cache = _prepare.cache
    out1 = [np.asarray(r["out1"], dtype=np.float64) for r in results]
    out2 = [np.asarray(r["out2"], dtype=np.float64) for r in results]

    # --- triplet ---
    qmax = np.empty(N)
    qmin = np.empty(N)
    for rt in range(32):
        R = rt // 4
        rows = slice(128 * rt, 128 * rt + 128)
        parts = []
        for cb in range(R, 8):                       # row-side partials
            c, t = TILE_AT[(rt, cb)]
            parts.append(out1[c][:, O_RMAX + t])
        for rp in range(0, 4 * R):                   # col-side partials
            c, t = TILE_AT[(rp, R)]
            parts.append(out2[c][:, 4 * (t - 4) + rt - 4 * R])
        qmax[rows] = np.max(np.stack(parts), axis=0)
        c, t = TILE_AT[(rt, R)]
        qmin[rows] = out1[c][:, O_RMIN + rt - 4 * R]

    d2_an = np.maximum(-2.0 * qmax, 1e-12)
    d2_ap = np.maximum(-2.0 * qmin - BIG, 1e-12)
    dist_an = np.sqrt(d2_an)
    dist_ap = np.sqrt(d2_ap)
    trip = np.mean(np.maximum(dist_ap - dist_an + MARGIN, 0.0))

    # --- cross entropy ---
    lse = np.empty(N)
    for c in range(NCORES):
        for x in range(4):
            lse[c * RPC + 128 * x: c * RPC + 128 * (x + 1)] = \
                out1[c][:, O_LSE + x]
    ti = cache["t_sorted"].astype(np.int64)
    ti = np.where(ti < 0, ti + C, ti)
    ti = np.clip(ti, 0, C - 1)
    gathered = cache["logits_p_bf"][np.arange(N), ti]
    xent = np.mean(lse - gathered)

    return np.float32(ALPHA * xent + BETA * trip)


def kernel(logits, feat, targets):
    from concourse.bass_utils import run_bass_kernel_spmd

    if "nc" not in _compiled:
        _compiled["nc"] = _build_nc()
    nc = _compiled["nc"]

    in_maps = _prepare(logits, feat, targets)
    res = run_bass_kernel_spmd(nc, in_maps, core_ids=list(range(NCORES)))
    return _combine(res.results)


# revision 38
# speedup vs baseline: 1.1682x; 1.1223x over previous
"""Trainium2 Bass kernel for combined cross-entropy + batch-hard triplet loss.

Problem (N=4096, C=751, D=2048, 1024 identities x 4 instances):
  loss = mean(-log_softmax(logits)[i, t_i]) +
         mean(relu(max_same(dist) - min_diff(dist) + 0.5))
  with dist = pairwise Euclidean distances of feat rows.

Strategy (measured ~68us on 8 NeuronCores, ~2.8x over a straightforward
row-parallel fp32 version):

* Symmetric tiling: the distance matrix is symmetric, so only upper-triangle
  tiles are computed -- at [128-row x 512-col] granularity 144 of 256 tiles,
  18 per core, balanced by a uniform slot pattern (4 diag + 4 + 2 + 8 tiles
  over at most two distinct 512-col blocks per core; SPMD addressing is
  identical on every core, per-core differences live in the input data).
* fp8 e4m3 everywhere on the PE with DoubleRow perf mode (~1.44x over
  bf16-rate at free-dim 512).  Features are quantized to fp8 on the host;
  the exact distance identity d2 = |f8_i - f8_j|^2 then holds for the
  quantized vectors, so the only error is a tiny input perturbation
  (~2e-4 relative on the final loss, gate is 2e-2).
* Per tile the PSUM accumulates
    psum = f8_i . f8_j - sq_i/2 - sq_j/2 - 8192*same(i,j)
  via 8 K=256 DoubleRow matmuls plus one fold matmul whose 40 contraction
  rows carry the same-pair mask (64 x -128) and a 4-level fp8 decomposition
  of -sq/32 against 16.0 (exact to ~0.03 in d2).
* Mining: row-side DVE max (hardest negative) on every tile, DVE min
  (hardest positive) on the 4 diagonal tiles; column-side coverage of the
  transposed region via ACT psum->SBUF f16 copy, PE transpose, DVE max.
* Warmup: the 4 diagonal tiles' lhsT is a slice of the already-resident
  slot0 rhs, so the PE starts after ~0.5 MB of DMA and runs k-major while
  the remaining slots stream in on three queues (SP=rhs+lhs, ACT=copies,
  gpsimd=fold/logits).  DMA issues are batched (~0.6us of engine time
  each).
* Cross entropy: device computes log-sum-exp per row (ACT Exp with fused
  accumulation, then Ln) over bf16 logits; host subtracts the gathered
  target logit.
* Host combines the small partial outputs (max over row/col-side partials
  -> dist_an, diagonal min -> dist_ap, sqrt/relu/mean in f64) and adds the
  xent term.  Rows are pre-sorted by target (the loss is permutation
  invariant) so same-identity groups are 4 consecutive rows, which makes
  the mask a fixed block pattern.
"""
import sys

if "/opt/trn_rl_repo" not in sys.path:
    sys.path.insert(0, "/opt/trn_rl_repo")

import numpy as np
import ml_dtypes

BF16 = ml_dtypes.bfloat16
FP8 = ml_dtypes.float8_e4m3

N = 4096
D = 2048
C = 751
NCORES = 8
RPC = N // NCORES          # xent rows per core = 512
NT = 18                    # distance tiles per core
KT = D // 128              # K chunks = 16
KF = 36                    # fold matmul contraction size
BIG = 16384.0              # same-pair offset in q = -2*psum
MASK_L = 64.0              # lhs mask scale; 64 * 128 = BIG/2 = 8192
MASK_R = 128.0             # rhs mask scale (max finite e4m3 is 240)
ALPHA = 1.0
BETA = 1.0
MARGIN = 0.5

# --- static tile assignment -------------------------------------------------
# Kept tiles: (row_tile r in 0..31, col_block c in 0..7) with r <= 4c+3.
# Slot sizes per core: [4, 4, 2, 8]; slot0 always holds the 4 diagonal-band
# tiles (r in 4c..4c+3). Each core touches at most 2 distinct col blocks.
SLOT_SIZES = [4, 4, 2, 8]
# tile positions: t0-3 slot0 (diag, warmup), t4-7 slot1, t8-15 slot3,
# t16-17 slot2 (the 2-tile slot last: its block can arrive when the bus
# is otherwise drained)
SLOT_OF_T = [0] * 4 + [1] * 4 + [2] * 2 + [3] * 8
ASSIGN = [
    [(0, [0, 1, 2, 3]), (7, [0, 1, 2, 3]), (7, [4, 5]), (7, list(range(6, 14)))],
    [(7, [28, 29, 30, 31]), (7, [14, 15, 16, 17]), (7, [18, 19]), (7, list(range(20, 28)))],
    [(1, [4, 5, 6, 7]), (1, [0, 1, 2, 3]), (6, [0, 1]), (6, list(range(2, 10)))],
    [(6, [24, 25, 26, 27]), (6, [10, 11, 12, 13]), (6, [14, 15]), (6, list(range(16, 24)))],
    [(2, [8, 9, 10, 11]), (5, [0, 1, 2, 3]), (5, [4, 5]), (2, list(range(0, 8)))],
    [(5, [20, 21, 22, 23]), (5, [6, 7, 8, 9]), (5, [10, 11]), (5, list(range(12, 20)))],
    [(3, [12, 13, 14, 15]), (3, [0, 1, 2, 3]), (4, [0, 1]), (3, list(range(4, 12)))],
    [(4, [16, 17, 18, 19]), (4, [2, 3, 4, 5]), (4, [6, 7]), (4, list(range(8, 16)))],
]

# TILES[c] = [(row_tile, col_block)] * 18, diag tiles at t=0..3
TILES = []
for _c in range(NCORES):
    _tl = []
    for _sl in (0, 1, 2, 3):
        _cb, _rows = ASSIGN[_c][_sl]
        _tl.extend((_r, _cb) for _r in _rows)
    TILES.append(_tl)

# SLOT_BLOCK[c][s] = col block resident in slot s for core c
SLOT_BLOCK = [[cb for cb, _ in ASSIGN[c]] for c in range(NCORES)]

# TILE_AT[(r, cb)] = (core, t)
TILE_AT = {}
for _c in range(NCORES):
    for _t, (_r, _cb) in enumerate(TILES[_c]):
        assert (_r, _cb) not in TILE_AT
        TILE_AT[(_r, _cb)] = (_c, _t)

# sanity: full upper-triangle coverage, diag placement
assert len(TILE_AT) == 144
for _r in range(32):
    for _cb in range(_r // 4, 8):
        assert (_r, _cb) in TILE_AT
DIAG_T = (0, 1, 2, 3)
for _c in range(NCORES):
    for _t, (_r, _cb) in enumerate(TILES[_c]):
        assert (_t in DIAG_T) == (_r // 4 == _cb), (_c, _t, _r, _cb)
        if _t in DIAG_T:
            _g = _t if _t < 3 else 3
            assert _r == 4 * _cb + _g  # diag lhsT = slot0 rhs slice

# out1 column layout
O_RMAX = 0                 # cols 0..17  : row-side max per tile
O_RMIN = NT                # cols 18..21 : row-side min, diag tiles t=0..3
O_LSE = NT + 4             # cols 22..25 : xent log-sum-exp per 128-row tile
O_W = NT + 8               # 26 cols

_compiled = {}


def _build_nc():
    import concourse.bass as bass  # noqa: F401
    import concourse.tile as tile
    from concourse import mybir, bacc
    from contextlib import ExitStack

    f32 = mybir.dt.float32
    f16 = mybir.dt.float16
    bf16 = mybir.dt.bfloat16
    fp8 = mybir.dt.float8e4
    PM = mybir.MatmulPerfMode.DoubleRow
    Alu = mybir.AluOpType
    Act = mybir.ActivationFunctionType
    X = mybir.AxisListType.X

    nc = bacc.Bacc("TRN2", target_bir_lowering=False, debug=False)

    # rhs_pack[s][p, j*1024 + i*512 + n]: slot s, DoubleRow pair j, subtile i
    rhs_in = nc.dram_tensor("rhs_pack", [4, 128, 8192], fp8, kind="ExternalInput").ap()
    # lhs_pack[u][p, tt*2048 + j*256 + i*128 + m]: tile pair u = tiles (4+2u, 5+2u)
    lhs_in = nc.dram_tensor("lhs_pack", [(NT - 4) // 2, 128, 4096], fp8, kind="ExternalInput").ap()
    flh_in = nc.dram_tensor("fold_lhs", [20, 2, NT * 128], fp8, kind="ExternalInput").ap()
    frh_in = nc.dram_tensor("fold_rhs", [NT, 20, 2, 512], fp8, kind="ExternalInput").ap()
    logits_in = nc.dram_tensor("logits", [128, 4 * C], bf16, kind="ExternalInput").ap()
    ident_in = nc.dram_tensor("ident", [128, 128], f16, kind="ExternalInput").ap()
    out1_dram = nc.dram_tensor("out1", [128, O_W], f32, kind="ExternalOutput").ap()
    out2_dram = nc.dram_tensor("out2", [128, (NT - 4) * 4], f32, kind="ExternalOutput").ap()

    with tile.TileContext(nc) as tc, ExitStack() as ctx:
        resident = ctx.enter_context(tc.tile_pool(name="resident", bufs=1))
        lhs_pool = ctx.enter_context(tc.tile_pool(name="lhs", bufs=(NT - 4) // 2))
        fr_pool = ctx.enter_context(tc.tile_pool(name="fr", bufs=NT))
        sb_pool = ctx.enter_context(tc.tile_pool(name="sbt", bufs=4))
        ps_pool = ctx.enter_context(tc.tile_pool(name="ps", bufs=6, space="PSUM"))
        pt_pool = ctx.enter_context(tc.tile_pool(name="pt", bufs=2, space="PSUM"))
        xent_pool = ctx.enter_context(tc.tile_pool(name="xent", bufs=2))
        small_pool = ctx.enter_context(tc.tile_pool(name="small", bufs=4))

        # --- resident loads.  SP queue carries rhs + lhs interleaved in
        # consumption order (warmup diag tiles read ONLY slot0 -- their lhsT
        # is a slice of the same resident data); gpsimd (SWDGE) carries
        # fold/logits/identity.  Few large DMAs: each dma_start costs the
        # issuing engine ~0.6us of descriptor generation.
        # slot0 in 4 groups of 2 chunk-pairs (fine warmup deps)
        R0_SIZES = (2, 2, 2, 2)
        r0g = []
        for g, npair in enumerate(R0_SIZES):
            rg = resident.tile([128, npair, 2, 512], fp8, tag=f"r0g{g}",
                               name=f"r0g{g}")
            r0g.append(rg)
        rs = [None] * 4   # slots 1..3 (slot1 in two halves)
        for ss in range(1, 4):
            rt = resident.tile([128, KT // 2, 2, 512], fp8, tag=f"rs{ss}",
                               name=f"rs{ss}")
            rs[ss] = rt
        rs1b = resident.tile([128, KT // 4, 2, 512], fp8, name="rs1b")
        lhs_pairs = []
        for u in range((NT - 4) // 2):
            lt = lhs_pool.tile([128, 2, KT // 2, 2, 128], fp8, tag="lhs")
            lhs_pairs.append(lt)

        # SP-queue emission order = arrival order: interleave the next slot
        # between slot0 warmup groups -- the (DMA-paced) warmup absorbs the
        # latency of rs1 loading behind it
        nc.sync.dma_start(r0g[0][:], rhs_in[0][:, 0:2048])
        nc.sync.dma_start(r0g[1][:], rhs_in[0][:, 2048:4096])
        nc.sync.dma_start(rs[1][:, 0:4], rhs_in[1][:, 0:4096])
        nc.sync.dma_start(r0g[2][:], rhs_in[0][:, 4096:6144])
        nc.sync.dma_start(rs1b[:], rhs_in[1][:, 4096:8192])
        nc.sync.dma_start(r0g[3][:], rhs_in[0][:, 6144:8192])
        nc.sync.dma_start(lhs_pairs[0][:], lhs_in[0])
        nc.sync.dma_start(lhs_pairs[1][:], lhs_in[1])
        nc.sync.dma_start(rs[2][:], rhs_in[2])
        nc.sync.dma_start(lhs_pairs[2][:], lhs_in[2])
        nc.sync.dma_start(rs[3][:], rhs_in[3])
        for u in range(3, (NT - 4) // 2):
            nc.sync.dma_start(lhs_pairs[u][:], lhs_in[u])

        ident = resident.tile([128, 128], f16)
        nc.gpsimd.dma_start(ident[:], ident_in[:])
        flh = resident.tile([20, 2, NT * 128], fp8)
        nc.gpsimd.dma_start(flh[:], flh_in[:])
        fr_tiles = []
        for t in range(NT):
            fr = fr_pool.tile([20, 2, 512], fp8, tag="fr")
            nc.gpsimd.dma_start(fr[:], frh_in[t])
            fr_tiles.append(fr)
        lg_all = resident.tile([128, 4 * C], bf16)
        nc.gpsimd.dma_start(lg_all[:], logits_in[:])

        out_tile = resident.tile([128, O_W], f32)
        out2_tile = resident.tile([128, (NT - 4) * 4], f32)

        R0_OF_J = []   # j -> (group, index within group)
        for g, npair in enumerate(R0_SIZES):
            R0_OF_J.extend((g, i) for i in range(npair))

        def rhs_op(s, j):
            if s == 0:
                g, i = R0_OF_J[j]
                return r0g[g][:, i]
            if s == 1 and j >= 4:
                return rs1b[:, j - 4]
            return rs[s][:, j]

        def diag_lhsT(j, g):
            gg, i = R0_OF_J[j]
            return r0g[gg][:, i, :, bass.ts(g, 128)]

        # --- warmup: k-major over the 4 diagonal tiles (slot0 only) ---
        ps_list = [ps_pool.tile([128, 512], f32, tag="ps", name=f"psw{_t}")
                   for _t in range(4)]
        for j in range(KT // 2):
            for t in range(4):
                nc.tensor.matmul(
                    ps_list[t][:],
                    diag_lhsT(j, t),
                    rhs_op(0, j),
                    start=(j == 0),
                    stop=False,
                    perf_mode=PM,
                )
        for t in range(4):
            nc.tensor.matmul(ps_list[t][:], flh[:, :, bass.ts(t, 128)],
                             fr_tiles[t][:], start=False, stop=True, perf_mode=PM)
            nc.vector.tensor_reduce(out_tile[:, O_RMAX + t:O_RMAX + t + 1],
                                    ps_list[t][:], axis=X, op=Alu.max)
            nc.vector.tensor_reduce(out_tile[:, O_RMIN + t:O_RMIN + t + 1],
                                    ps_list[t][:], axis=X, op=Alu.min)

        # --- cross entropy: lse per row (grouped per activation function
        # to avoid ACT table reloads) ---
        saccs = []
        for x in range(4):
            escr = xent_pool.tile([128, C], bf16, tag="escr")
            sacc = small_pool.tile([128, 1], f32, tag="s")
            nc.scalar.activation(escr[:], lg_all[:, bass.ts(x, C)], Act.Exp,
                                 accum_out=sacc[:])
            saccs.append(sacc)
        for x in range(4):
            nc.scalar.activation(out_tile[:, O_LSE + x:O_LSE + x + 1], saccs[x][:],
                                 Act.Ln, scale=1.0)

        # --- main loop: tiles 4..17, software-pipelined transposes ---
        sb_tiles = [None] * NT

        def emit_transpose(t):
            sb = sb_tiles[t]
            pt = pt_pool.tile([128, 4, 128], f16, tag="pt")
            for i in range(4):
                nc.tensor.transpose(pt[:, i, :], sb[:, bass.ts(i, 128)], ident[:])
            nc.vector.tensor_reduce(out2_tile[:, bass.ts(t - 4, 4)], pt[:],
                                    axis=X, op=Alu.max)

        # tiles processed in pairs with interleaved matmul streams: the PE
        # reorder window can overlap the two banks' weight loads
        for ta in range(4, NT, 2):
            tb = ta + 1
            psa = ps_pool.tile([128, 512], f32, tag="ps", name=f"psa{ta}")
            psb = ps_pool.tile([128, 512], f32, tag="ps", name=f"psb{ta}")
            for j in range(KT // 2):
                for t, ps in ((ta, psa), (tb, psb)):
                    lhsT = lhs_pairs[(t - 4) // 2][:, (t - 4) % 2, j]
                    nc.tensor.matmul(ps[:], lhsT, rhs_op(SLOT_OF_T[t], j),
                                     start=(j == 0), stop=False, perf_mode=PM)
            for t, ps in ((ta, psa), (tb, psb)):
                nc.tensor.matmul(ps[:], flh[:, :, bass.ts(t, 128)],
                                 fr_tiles[t][:], start=False, stop=True,
                                 perf_mode=PM)
            for t, ps in ((ta, psa), (tb, psb)):
                nc.vector.tensor_reduce(out_tile[:, O_RMAX + t:O_RMAX + t + 1],
                                        ps[:], axis=X, op=Alu.max)
                sb = sb_pool.tile([128, 512], f16, tag="sb")
                nc.scalar.activation(sb[:], ps[:], Act.Copy, scale=1.0)
                sb_tiles[t] = sb
            # transposes of the previous pair land behind this pair's matmuls
            if ta >= 6:
                emit_transpose(ta - 2)
                emit_transpose(ta - 1)
        emit_transpose(NT - 2)
        emit_transpose(NT - 1)

        nc.sync.dma_start(out2_dram[:], out2_tile[:])
        nc.sync.dma_start(out1_dram[:], out_tile[:])

    nc.compile()
    return nc


def _prepare(logits, feat, targets):
    logits = np.asarray(logits, dtype=np.float32)
    feat = np.asarray(feat, dtype=np.float32)
    targets = np.asarray(targets)

    perm = np.argsort(targets, kind="stable")
    t_sorted = np.asarray(targets)[perm]
    tg = t_sorted.reshape(-1, 4)
    assert (tg == tg[:, :1]).all(), "expected PK sampling with 4 instances/identity"

    F = feat[perm].astype(FP8)                   # [N, D] fp8 e4m3
    FT = np.ascontiguousarray(F.T)               # [D, N] fp8
    F64 = F.astype(np.float64)
    sq = np.einsum("ij,ij->i", F64, F64).astype(np.float32)
    # 4-level fp8 decomposition of -sq/32 (e4m3 max is 448; the factor 16
    # sits on the "ones" side so each product contributes -sq/2 overall)
    sq_lv = []
    res = (sq / -32.0).astype(np.float32)
    for _ in range(4):
        lv = res.astype(FP8).astype(np.float32)
        sq_lv.append(lv)
        res = res - lv

    logits_p = logits[perm].astype(BF16)

    # mask patterns (bf16-exact values, stored f32 then cast)
    mask_lhs = np.zeros((32, 128), dtype=np.float32)
    m_idx = np.arange(128)
    mask_lhs[m_idx // 4, m_idx] = MASK_L

    FT3 = FT.reshape(KT, 128, N)
    FT4 = FT.reshape(KT // 2, 2, 128, N)

    in_maps = []
    for c in range(NCORES):
        tiles = TILES[c]

        # rhs_pack [4, 128, 8192]: [s][p, j*1024 + i*512 + n],
        # slot s holds block SLOT_BLOCK[c][s]
        rhs_pack = np.empty((4, 128, 8192), dtype=FP8)
        for sl in range(4):
            cb0 = 512 * SLOT_BLOCK[c][sl]
            blk = FT4[:, :, :, cb0:cb0 + 512]           # [KT//2 j, 2 i, 128p, 512]
            rhs_pack[sl] = blk.transpose(2, 0, 1, 3).reshape(128, 8192)

        # lhs_pack [(NT-4)//2, 128, 4096]: tile pair u = (4+2u, 5+2u),
        # [u][p, tt*2048 + j*256 + i*128 + m] = FT[128*(2j+i)+p, rows_t[m]]
        # (diag tiles t<4 read their lhsT out of the resident slot0 rhs)
        lhs_pack = np.empty(((NT - 4) // 2, 128, 4096), dtype=FP8)
        for t, (r, _cb) in enumerate(tiles):
            if t < 4:
                continue
            blk = FT3[:, :, 128 * r:128 * r + 128]      # [KT, 128p, 128m]
            u, tt = (t - 4) // 2, (t - 4) % 2
            lhs_pack[u][:, 2048 * tt:2048 * (tt + 1)] = \
                blk.transpose(1, 0, 2).reshape(128, 2048)

        # fp8 fold: 40 logical contraction rows = 32 mask + 4 row-sq levels
        # + 4 col-sq levels, packed as DoubleRow [20, 2, .] (row k -> (k%20,
        # k//20)).  sq levels are a 4-term fp8 decomposition of -sq/2 (exact
        # to ~0.016).
        flh40 = np.zeros((40, NT * 128), dtype=np.float32)
        frh40 = np.zeros((NT, 40, 512), dtype=np.float32)
        for t, (r, cb) in enumerate(tiles):
            cs = slice(128 * t, 128 * t + 128)
            flh40[:32, cs] = mask_lhs
            rows = slice(128 * r, 128 * r + 128)
            for lv in range(4):
                flh40[32 + lv, cs] = sq_lv[lv][rows]     # row-sq levels
                flh40[36 + lv, cs] = 16.0                # x16 for col-sq
            cols_blk = slice(512 * cb, 512 * cb + 512)
            for lv in range(4):
                frh40[t, 32 + lv] = 16.0                 # x16 for row-sq
                frh40[t, 36 + lv] = sq_lv[lv][cols_blk]  # col-sq levels
            if r // 4 == cb:  # diagonal-band tile: same-pair mask
                base = 128 * (r % 4)
                for g in range(32):
                    frh40[t, g, base + 4 * g: base + 4 * g + 4] = -MASK_R
        flh = flh40.reshape(2, 20, NT * 128).transpose(1, 0, 2)
        frh = frh40.reshape(NT, 2, 20, 512).transpose(0, 2, 1, 3)

        lgp = logits_p[c * RPC:(c + 1) * RPC]       # [512, C] bf16
        lg_pack = np.ascontiguousarray(
            lgp.reshape(4, 128, C).transpose(1, 0, 2).reshape(128, 4 * C))

        in_maps.append({
            "rhs_pack": rhs_pack,
            "lhs_pack": lhs_pack,
            "fold_lhs": np.ascontiguousarray(flh).astype(FP8),
            "fold_rhs": np.ascontiguousarray(frh).astype(FP8),
            "logits": lg_pack,
            "ident": np.eye(128, dtype=np.float16),
        })

    # stash for _combine
    _prepare.cache = {
        "logits_p_bf": logits_p.astype(np.float64),
        "t_sorted": t_sorted,
    }
    return in_maps


def _combine(results):
    cache = _prepare.cache
    out1 = [np.asarray(r["out1"], dtype=np.float64) for r in results]
    out2 = [np.asarray(r["out2"], dtype=np.float64) for r in results]

    # --- triplet ---
    qmax = np.empty(N)
    qmin = np.empty(N)
    for rt in range(32):
        R = rt // 4
        rows = slice(128 * rt, 128 * rt + 128)
        parts = []
        for cb in range(R, 8):                       # row-side partials
            c, t = TILE_AT[(rt, cb)]
            parts.append(out1[c][:, O_RMAX + t])
        for rp in range(0, 4 * R):                   # col-side partials
            c, t = TILE_AT[(rp, R)]
            parts.append(out2[c][:, 4 * (t - 4) + rt - 4 * R])
        qmax[rows] = np.max(np.stack(parts), axis=0)
        c, t = TILE_AT[(rt, R)]
        qmin[rows] = out1[c][:, O_RMIN + rt - 4 * R]

    d2_an = np.maximum(-2.0 * qmax, 1e-12)
    d2_ap = np.maximum(-2.0 * qmin - BIG, 1e-12)
    dist_an = np.sqrt(d2_an)
    dist_ap = np.sqrt(d2_ap)
    trip = np.mean(np.maximum(dist_ap - dist_an + MARGIN, 0.0))

    # --- cross entropy ---
    lse = np.empty(N)
    for c in range(NCORES):
        for x in range(4):
            lse[c * RPC + 128 * x: c * RPC + 128 * (x + 1)] = \
                out1[c][:, O_LSE + x]
    ti = cache["t_sorted"].astype(np.int64)
    ti = np.where(ti < 0, ti + C, ti)
    ti = np.clip(ti, 0, C - 1)
    gathered = cache["logits_p_bf"][np.arange(N), ti]
    xent = np.mean(lse - gathered)

    return np.float32(ALPHA * xent + BETA * trip)


def kernel(logits, feat, targets):
    from concourse.bass_utils import run_bass_kernel_spmd

    if "nc" not in _compiled:
        _compiled["nc"] = _build_nc()
    nc = _compiled["nc"]

    in_maps = _prepare(logits, feat, targets)
    res = run_bass_kernel_spmd(nc, in_maps, core_ids=list(range(NCORES)))
    return _combine(res.results)
